# revision 4
# baseline (speedup 1.0000x reference)
"""Trainium2 Bass kernel: ConvTranspose3d(32->64,k3,s2,p1) + 0.5x + MaxPool3d(2) +
global-avg-pool + clamp(0,1), data-parallel over batch on 8 NeuronCores.

Math: a stride-2 transposed conv splits into 8 parity classes (even/odd output
index per spatial axis); each 2x2x2 maxpool window holds exactly one output of
each class, so maxpool == elementwise max over the 8 class sub-convolutions.

Conv: fp8(e4m3) DoubleRow matmuls contract K=256 per pass = 4 partition blocks
((dh,dw) shifted copies of x) x 32 c_in x 2 ko k-tiles (the dd shift, a d-axis
stride in the rhs AP).  The full {0,1}^3 shift cube is addressable in one
matmul, so each class's complete tap set fits a single pass: 4 passes x
(2 classes per 128-partition PSUM output) cover all 8 classes at 0.5
cycles/row.  Weights are pre-scaled by 128 into fp8 range; the final mean
scale divides it back out.

Consume (max over 8 classes + mean), balanced over ACT/DVE/Pool:
- ACT evacuates each pair's PC tile (4 slots, both chunks merged into one
  [128,4,496] op; the off-chunk tail columns are garbage that nothing reads).
- DVE maxes the evacuated slots against the PM tile directly in PSUM (1x) for
  normal pairs; heavy pairs ACT-evacuate PM too and the max runs all-SBUF 2x.
- L2 slot-pair maxes run per chunk on DVE, or decomposed as sub/relu/add on
  the otherwise-idle Pool engine (which lacks a tensor max).
- Cross-half folds are batched two pairs at a time: DMAs stack two pairs' mm
  halves into [128, 961] tiles so the fold max + fused sum-accumulate use all
  128 partitions; a tiny end-of-kernel partition fold merges the two halves.
"""

import numpy as np

import concourse.bass as bass
import concourse.bacc as bacc
import concourse.mybir as mybir
from concourse.tile import TileContext
from concourse.bass_utils import run_bass_kernel_spmd
from concourse.alu_op_type import AluOpType

# Problem constants (hardcoded per contract)
N_BATCH = 8
IN_C, OUT_C = 32, 64
D, H, W = 16, 32, 32
JD, JH, JW = 15, 31, 31          # pooled output grid
NPOS = JD * JH * JW              # 14415
SCALE = 0.5
WS = 128.0                       # fp8 weight pre-scale
FREE = D * H * W                 # 16384 flat free size per c_in
DSTR, HSTR = H * W, W            # flat strides

BLOCKS = [(0, 0), (0, 1), (1, 0), (1, 1)]          # (dh, dw) per 32-row block
BLOCK_OFF = [dh * HSTR + dw for (dh, dw) in BLOCKS]

CHUNKS = [(jd, h0, hcnt) for jd in range(JD) for (h0, hcnt) in ((0, 16), (16, 15))]
CN = [hcnt * JW for (_, _, hcnt) in CHUNKS]
PAIRW = CN[0] + CN[1]   # 961 cols per pair tile
NPAIR = len(CHUNKS) // 2
NPASS = 4

# Classes (pd, ph, pw); pass p computes CLS[p] as PSUM partition halves.
CLS = [
    ((0, 0, 0), (1, 1, 1)),
    ((0, 0, 1), (1, 1, 0)),
    ((0, 1, 0), (1, 0, 1)),
    ((0, 1, 1), (1, 0, 0)),
]

# Engine-balance knobs (cost-model tuned).
HEAVY_PAIRS = {2, 5, 8, 11, 14}       # pairs whose PM is ACT-evacuated too
POOL_L2 = {0, 2, 4, 6, 8, 10, 12, 14, 16, 18, 20, 22, 24, 26, 28}  # chunks w/ Pool L2


def build_wstack(w: np.ndarray) -> np.ndarray:
    """Stack torch-layout ConvTranspose3d weights (in,out,kd,kh,kw) into the
    4 DoubleRow lhsT blocks, one [128, 4*2*128] array: rows = 32*block + c_in;
    cols = 256*pass + 128*ko + 64*half + c_out.  Unused slots stay 0."""
    wstk = np.zeros((128, NPASS * 2 * 128), np.float32)
    for p in range(NPASS):
        for half, (pd, ph, pw) in enumerate(CLS[p]):
            for ko in range(2):
                if pd == 0 and ko == 1:
                    continue
                kd = 1 if pd == 0 else 2 - 2 * ko
                for bidx, (dh, dw) in enumerate(BLOCKS):
                    if dh > ph or dw > pw:
                        continue
                    kh = 1 if ph == 0 else 2 - 2 * dh
                    kw = 1 if pw == 0 else 2 - 2 * dw
                    col = p * 256 + ko * 128 + half * 64
                    wstk[32 * bidx: 32 * bidx + 32, col: col + OUT_C] = (
                        w[:, :, kd, kh, kw] * WS
                    )
    return wstk


def build_nc() -> bass.Bass:
    nc = bacc.Bacc()
    f32 = mybir.dt.float32
    bf16 = mybir.dt.bfloat16
    fp8 = mybir.dt.float8e4

    x_d = nc.declare_dram_parameter("x", [IN_C, FREE], fp8, isOutput=False)
    w_d = nc.declare_dram_parameter("wstk", [128, NPASS * 256], fp8, isOutput=False)
    b_d = nc.declare_dram_parameter("bvec", [OUT_C, 1], f32, isOutput=False)
    o_d = nc.declare_dram_parameter("out", [OUT_C, 1], f32, isOutput=True)

    with TileContext(nc) as tc:
        with (
            tc.tile_pool(name="xp", bufs=1) as xp,
            tc.tile_pool(name="wp", bufs=1) as wp,
            tc.tile_pool(name="ps", bufs=2, space="PSUM") as ps,
            tc.tile_pool(name="mp", bufs=3) as mp,
            tc.tile_pool(name="fp", bufs=2) as fpool,
            tc.tile_pool(name="ap", bufs=1) as ap,
        ):
            # Trigger the ACT table load at t=0 so it overlaps the x DMA.
            warm = ap.tile([1, 1], bf16, tag="warm")
            nc.vector.memset(warm[:, :], 0.0)
            nc.scalar.copy(warm[:, :], warm[:, :])

            wt = wp.tile([128, NPASS * 256], fp8, tag="wt")
            nc.sync.dma_start(out=wt[:, :], in_=w_d[:, :])
            bv = wp.tile([OUT_C, 1], f32, tag="bv")
            nc.sync.dma_start(out=bv[:, :], in_=b_d[:, :])

            xbuf = xp.tile([128, FREE], fp8, tag="x")
            SLAB = 2 * DSTR
            HALF = FREE // 2
            for bidx, off in enumerate(BLOCK_OFF):
                nc.sync.dma_start(
                    out=xbuf[32 * bidx: 32 * bidx + 32, 0:SLAB],
                    in_=x_d[:, off: off + SLAB],
                )
            for bidx, off in enumerate(BLOCK_OFF):
                nc.sync.dma_start(
                    out=xbuf[32 * bidx: 32 * bidx + 32, SLAB:HALF],
                    in_=x_d[:, SLAB + off: off + HALF],
                )
            for bidx, off in enumerate(BLOCK_OFF):
                ln = FREE - HALF - 33
                nc.sync.dma_start(
                    out=xbuf[32 * bidx: 32 * bidx + 32, HALF: HALF + ln],
                    in_=x_d[:, HALF + off: HALF + off + ln],
                )

            xv = xbuf[:, :].rearrange("p (d h w) -> p d h w", d=D, h=H, w=W)
            wtv = wt[:, :].rearrange("k (p ko m) -> k p ko m", p=NPASS, ko=2)
            # acc column c: partition p<64 = pair-group even sums, p>=64 = odd
            acc = ap.tile([128, NPAIR // 2 + 1], f32, tag="acc")
            # the last fold group only writes partitions 0:64 of its column
            nc.vector.memset(acc[:, :], 0.0)

            def consume_pair(pi, PC, PM, mm):
                """8 classes x 2 chunks -> mm [128, 961] (max of each half's 4)."""
                cbase = 2 * pi
                PCv = PC[:, :].rearrange("p (s n) -> p s n", s=4, n=512)
                PMv = PM[:, :].rearrange("p (s n) -> p s n", s=4, n=512)
                cc = mp.tile([128, 4, 496], bf16, name="cc", tag="cc")
                nc.scalar.copy(cc[:, :, :], PCv[:, :, 0:496])
                m = mp.tile([128, 4, 496], bf16, name="m", tag="m")
                if pi in HEAVY_PAIRS:
                    cm = mp.tile([128, 4, 496], bf16, name="cm", tag="cm")
                    nc.scalar.copy(cm[:, :, :], PMv[:, :, 0:496])
                    nc.vector.tensor_max(m[:, :, :], cc[:, :, :], cm[:, :, :])
                else:
                    nc.vector.tensor_max(m[:, :, :], cc[:, :, :], PMv[:, :, 0:496])
                for k in range(2):
                    ci = cbase + k
                    n = CN[ci]
                    o = 0 if k == 0 else CN[cbase]
                    if ci in POOL_L2:
                        dd = mp.tile([128, 496], bf16, name="dd", tag="dd")
                        rr = mp.tile([128, 496], bf16, name="rr", tag="rr")
                        nc.gpsimd.tensor_sub(
                            dd[:, 0:n], m[:, 2 * k, 0:n], m[:, 2 * k + 1, 0:n]
                        )
                        nc.gpsimd.tensor_relu(rr[:, 0:n], dd[:, 0:n])
                        nc.gpsimd.tensor_add(
                            mm[:, o: o + n], m[:, 2 * k + 1, 0:n], rr[:, 0:n]
                        )
                    else:
                        nc.vector.tensor_max(
                            mm[:, o: o + n], m[:, 2 * k, 0:n], m[:, 2 * k + 1, 0:n]
                        )

            def fold_group(gi, mms):
                """Stack 1-2 pairs' mm halves into [*, 961] tiles via DMA and
                fold + accumulate once for the whole group."""
                np_ = len(mms)
                rows = 64 * np_
                lo = fpool.tile([128, PAIRW], bf16, name="lo", tag="lo")
                hi = fpool.tile([128, PAIRW], bf16, name="hi", tag="hi")
                for j, mmj in enumerate(mms):
                    nc.sync.dma_start(
                        out=lo[64 * j: 64 * j + 64, :], in_=mmj[0:OUT_C, :]
                    )
                    nc.sync.dma_start(
                        out=hi[64 * j: 64 * j + 64, :], in_=mmj[OUT_C:128, :]
                    )
                mh = fpool.tile([128, PAIRW], bf16, name="mh", tag="mh")
                mg = fpool.tile([128, PAIRW], bf16, name="mg", tag="mg")
                nc.vector.tensor_max(mh[0:rows, :], lo[0:rows, :], hi[0:rows, :])
                nc.vector.tensor_scalar(
                    mg[0:rows, :], mh[0:rows, :], 1.0, None,
                    op0=AluOpType.mult, op1=AluOpType.add,
                    accum_out=acc[0:rows, gi: gi + 1],
                )

            pending = []
            group = []
            gi = 0
            for pi in range(NPAIR):
                pair = CHUNKS[2 * pi: 2 * pi + 2]
                PC = ps.tile([128, 4 * 512], f32, name="PC", tag="bp")
                PM = ps.tile([128, 4 * 512], f32, name="PM", tag="bp")
                for tile_i, bp in enumerate((PC, PM)):
                    for s in range(2):
                        p = 2 * tile_i + s
                        for k, (jd, h0, hcnt) in enumerate(pair):
                            n = hcnt * JW
                            nc.tensor.matmul(
                                bp[:, :].rearrange(
                                    "p (s n) -> p s n", s=4, n=512
                                )[:, 2 * k + s, 0:n],
                                wtv[:, p, :, :],
                                xv[:, jd: jd + 2, h0: h0 + hcnt, 0:JW],
                                start=True,
                                stop=True,
                                perf_mode=mybir.MatmulPerfMode.DoubleRow,
                            )
                mm = mp.tile([128, PAIRW], bf16, name="mm", tag="mm")
                consume_pair(pi, PC, PM, mm)
                group.append(mm)
                if len(group) == 2 or pi == NPAIR - 1:
                    if len(pending) >= 2:
                        fold_group(*pending.pop(0))
                    pending.append((gi, group))
                    group = []
                    gi += 1
            while pending:
                fold_group(*pending.pop(0))

            # total: sum acc columns, then fold partition halves via DMA.
            tot = ap.tile([128, 1], f32, tag="tot")
            nc.vector.reduce_sum(tot[:, :], acc[:, :], axis=mybir.AxisListType.X)
            tots = ap.tile([OUT_C, 1], f32, tag="tots")
            nc.sync.dma_start(out=tots[:, :], in_=tot[OUT_C:128, :])
            tot2 = ap.tile([OUT_C, 1], f32, tag="tot2")
            nc.vector.tensor_add(tot2[:, :], tot[0:OUT_C, :], tots[:, :])
            res = ap.tile([OUT_C, 1], f32, tag="res")
            # res = tot2 * (SCALE/(NPOS*WS)) + 0.5*b; bvec is pre-scaled.
            nc.vector.scalar_tensor_tensor(
                res[:, :], tot2[:, :], SCALE / (NPOS * WS), bv[:, :],
                op0=AluOpType.mult, op1=AluOpType.add,
            )
            out_t = ap.tile([OUT_C, 1], f32, tag="outt")
            nc.vector.tensor_scalar(
                out_t[:, :], res[:, :], 0.0, 1.0,
                op0=AluOpType.max, op1=AluOpType.min,
            )
            nc.sync.dma_start(out=o_d[:, :], in_=out_t[:, :])

    return nc


_NC_CACHE = None


def _get_nc():
    global _NC_CACHE
    if _NC_CACHE is None:
        _NC_CACHE = build_nc()
        _NC_CACHE.finalize()
    return _NC_CACHE


def run(x, w, b, **spmd_kwargs):
    """Run on 8 cores; returns (output (8,64,1,1,1), BassKernelResults)."""
    import ml_dtypes
    f8 = np.dtype(ml_dtypes.float8_e4m3)
    x = np.ascontiguousarray(x, np.float32)
    wstk = build_wstack(np.asarray(w, np.float32)).astype(f8)
    bvec = (SCALE * np.asarray(b, np.float32)).reshape(OUT_C, 1).copy()
    nc = _get_nc()
    in_maps = [
        {"x": x[i].reshape(IN_C, FREE).astype(f8), "wstk": wstk, "bvec": bvec}
        for i in range(N_BATCH)
    ]
    r = run_bass_kernel_spmd(nc, in_maps, list(range(N_BATCH)), **spmd_kwargs)
    out = np.stack(
        [np.asarray(r.results[i]["out"], np.float32).reshape(OUT_C) for i in range(N_BATCH)]
    )
    return out.reshape(N_BATCH, OUT_C, 1, 1, 1), r


def kernel(x, w, b):
    out, _ = run(x, w, b)
    return out


# revision 6
# speedup vs baseline: 1.0635x; 1.0635x over previous
"""Trainium2 Bass kernel: ConvTranspose3d(32->64,k3,s2,p1) + 0.5x + MaxPool3d(2) +
global-avg-pool + clamp(0,1), data-parallel over batch on 8 NeuronCores.

Math: a stride-2 transposed conv splits into 8 parity classes (even/odd output
index per spatial axis); each 2x2x2 maxpool window holds exactly one output of
each class, so maxpool == elementwise max over the 8 class sub-convolutions.

Conv: fp8(e4m3) DoubleRow matmuls contract K=256 per pass = 4 partition blocks
((dh,dw) shifted copies of x) x 32 c_in x 2 ko k-tiles (the dd shift, a d-axis
stride in the rhs AP).  The full {0,1}^3 shift cube is addressable in one
matmul, so each class's complete tap set fits a single pass: 4 passes x
(2 classes per 128-partition PSUM output) cover all 8 classes at 0.5
cycles/row.  Weights are pre-scaled by 128 into fp8 range; the final mean
scale divides it back out.

Consume (max over 8 classes + mean): pairs alternate mixed/heavy so ACT and
DVE stay balanced: ACT always evacuates the PC tile (one merged [128,4,496]
copy per pair; off-chunk tail columns are garbage nothing reads), and on heavy
pairs PM as well, making the first-level max an all-SBUF bf16 2x DVE op;
mixed pairs max the evacuated PC against PM directly in PSUM (1x).  The
cross-half folds batch two pairs: DMAs stack both pairs' mm halves so the
fold max + fused sum-accumulate use all 128 partitions (pair j's lo half is
DMA'd into pair i's mm tile after its hi half has been copied out).
"""

import numpy as np

import concourse.bass as bass
import concourse.bacc as bacc
import concourse.mybir as mybir
from concourse.tile import TileContext
from concourse.bass_utils import run_bass_kernel_spmd
from concourse.alu_op_type import AluOpType

# Problem constants (hardcoded per contract)
N_BATCH = 8
IN_C, OUT_C = 32, 64
D, H, W = 16, 32, 32
JD, JH, JW = 15, 31, 31          # pooled output grid
NPOS = JD * JH * JW              # 14415
SCALE = 0.5
WS = 128.0                       # fp8 weight pre-scale
FREE = D * H * W                 # 16384 flat free size per c_in
DSTR, HSTR = H * W, W            # flat strides

BLOCKS = [(0, 0), (0, 1), (1, 0), (1, 1)]          # (dh, dw) per 32-row block
BLOCK_OFF = [dh * HSTR + dw for (dh, dw) in BLOCKS]

CHUNKS = [(jd, h0, hcnt) for jd in range(JD) for (h0, hcnt) in ((0, 16), (16, 15))]
CN = [hcnt * JW for (_, _, hcnt) in CHUNKS]
PAIRW = CN[0] + CN[1]   # 961 cols per pair tile
NPAIR = len(CHUNKS) // 2
NGROUP = (NPAIR + 1) // 2
NPASS = 4

# Classes (pd, ph, pw); pass p computes CLS[p] as PSUM partition halves.
CLS = [
    ((0, 0, 0), (1, 1, 1)),
    ((0, 0, 1), (1, 1, 0)),
    ((0, 1, 0), (1, 0, 1)),
    ((0, 1, 1), (1, 0, 0)),
]

HEAVY_PAIRS = {1, 3, 5, 7, 9, 11, 13}   # PM evacuated too; L1 runs all-SBUF 2x


def build_wstack(w: np.ndarray) -> np.ndarray:
    """Stack torch-layout ConvTranspose3d weights (in,out,kd,kh,kw) into the
    4 DoubleRow lhsT blocks, one [128, 4*2*128] array: rows = 32*block + c_in;
    cols = 256*pass + 128*ko + 64*half + c_out.  Unused slots stay 0."""
    wstk = np.zeros((128, NPASS * 2 * 128), np.float32)
    for p in range(NPASS):
        for half, (pd, ph, pw) in enumerate(CLS[p]):
            for ko in range(2):
                if pd == 0 and ko == 1:
                    continue
                kd = 1 if pd == 0 else 2 - 2 * ko
                for bidx, (dh, dw) in enumerate(BLOCKS):
                    if dh > ph or dw > pw:
                        continue
                    kh = 1 if ph == 0 else 2 - 2 * dh
                    kw = 1 if pw == 0 else 2 - 2 * dw
                    col = p * 256 + ko * 128 + half * 64
                    wstk[32 * bidx: 32 * bidx + 32, col: col + OUT_C] = (
                        w[:, :, kd, kh, kw] * WS
                    )
    return wstk


def build_nc() -> bass.Bass:
    nc = bacc.Bacc()
    f32 = mybir.dt.float32
    bf16 = mybir.dt.bfloat16
    fp8 = mybir.dt.float8e4

    x_d = nc.declare_dram_parameter("x", [IN_C, FREE], fp8, isOutput=False)
    w_d = nc.declare_dram_parameter("wstk", [128, NPASS * 256], fp8, isOutput=False)
    b_d = nc.declare_dram_parameter("bvec", [OUT_C, 1], f32, isOutput=False)
    o_d = nc.declare_dram_parameter("out", [OUT_C, 1], f32, isOutput=True)

    with TileContext(nc) as tc:
        with (
            tc.tile_pool(name="xp", bufs=1) as xp,
            tc.tile_pool(name="wp", bufs=1) as wp,
            tc.tile_pool(name="ps", bufs=2, space="PSUM") as ps,
            tc.tile_pool(name="mp", bufs=3) as mp,
            tc.tile_pool(name="fp", bufs=2) as fpool,
            tc.tile_pool(name="ap", bufs=1) as ap,
        ):
            # Trigger the ACT table load at t=0 so it overlaps the x DMA.
            warm = ap.tile([1, 1], bf16, tag="warm")
            nc.gpsimd.memset(warm[:, :], 0.0)
            nc.scalar.copy(warm[:, :], warm[:, :])

            wt = wp.tile([128, NPASS * 256], fp8, tag="wt")
            nc.sync.dma_start(out=wt[:, :], in_=w_d[:, :])
            bv = wp.tile([OUT_C, 1], f32, tag="bv")
            nc.gpsimd.dma_start(out=bv[:, :], in_=b_d[:, :])

            xbuf = xp.tile([128, FREE], fp8, tag="x")
            # 4 shifted copies; priority slab (d-rows 0-1) per block first so
            # the first chunk pair's matmuls start early.  Split dispatch
            # between HWDGE (sync) and SWDGE (gpsimd) to halve queue serial.
            SLAB = 2 * DSTR
            HALF = FREE // 2
            for bidx, off in enumerate(BLOCK_OFF):
                eng = nc.sync if bidx % 2 == 0 else nc.gpsimd
                eng.dma_start(
                    out=xbuf[32 * bidx: 32 * bidx + 32, 0:SLAB],
                    in_=x_d[:, off: off + SLAB],
                )
            for bidx, off in enumerate(BLOCK_OFF):
                eng = nc.sync if bidx % 2 == 1 else nc.gpsimd
                eng.dma_start(
                    out=xbuf[32 * bidx: 32 * bidx + 32, SLAB:HALF],
                    in_=x_d[:, SLAB + off: off + HALF],
                )
            for bidx, off in enumerate(BLOCK_OFF):
                ln = FREE - HALF - 33
                eng = nc.sync if bidx % 2 == 0 else nc.gpsimd
                eng.dma_start(
                    out=xbuf[32 * bidx: 32 * bidx + 32, HALF: HALF + ln],
                    in_=x_d[:, HALF + off: HALF + off + ln],
                )

            xv = xbuf[:, :].rearrange("p (d h w) -> p d h w", d=D, h=H, w=W)
            wtv = wt[:, :].rearrange("k (p ko m) -> k p ko m", p=NPASS, ko=2)
            # acc column g: partition p<64 = group g's even pair, p>=64 = odd
            acc = ap.tile([128, NGROUP], f32, tag="acc")
            nc.vector.memset(acc[:, :], 0.0)

            def consume_pair(pi, PC, PM, mm):
                """8 classes x 2 chunks -> mm [128, 961] (max of each half's 4)."""
                cbase = 2 * pi
                PCv = PC[:, :].rearrange("p (s n) -> p s n", s=4, n=512)
                PMv = PM[:, :].rearrange("p (s n) -> p s n", s=4, n=512)
                cc = mp.tile([128, 4, 496], bf16, name="cc", tag="cc")
                nc.scalar.copy(cc[:, :, :], PCv[:, :, 0:496])
                m = mp.tile([128, 4, 496], bf16, name="m", tag="m")
                if pi in HEAVY_PAIRS:
                    cm = mp.tile([128, 4, 496], bf16, name="cm", tag="cm")
                    nc.scalar.copy(cm[:, :, :], PMv[:, :, 0:496])
                    nc.vector.tensor_max(m[:, :, :], cc[:, :, :], cm[:, :, :])
                else:
                    nc.vector.tensor_max(m[:, :, :], cc[:, :, :], PMv[:, :, 0:496])
                for k in range(2):
                    ci = cbase + k
                    n = CN[ci]
                    o = 0 if k == 0 else CN[cbase]
                    nc.vector.tensor_max(
                        mm[:, o: o + n], m[:, 2 * k, 0:n], m[:, 2 * k + 1, 0:n]
                    )

            def fold_group(gi, mms):
                """Fold 1-2 pairs' cross-half maxes + accumulate.  For a
                2-pair group, pair j's halves are DMA'd so pair i's mm tile
                becomes the lo stack (j's lo overwrites i's hi after the hi
                stack has copied it out)."""
                if len(mms) == 2:
                    mmi, mmj = mms
                    hi = fpool.tile([128, PAIRW], bf16, name="hi", tag="hi")
                    nc.sync.dma_start(out=hi[0:64, :], in_=mmi[OUT_C:128, :])
                    nc.sync.dma_start(out=hi[64:128, :], in_=mmj[OUT_C:128, :])
                    nc.sync.dma_start(out=mmi[OUT_C:128, :], in_=mmj[0:OUT_C, :])
                    lo, rows = mmi, 128
                else:
                    hi = fpool.tile([128, PAIRW], bf16, name="hi", tag="hi")
                    nc.sync.dma_start(out=hi[0:64, :], in_=mms[0][OUT_C:128, :])
                    lo, rows = mms[0], 64
                mh = fpool.tile([128, PAIRW], bf16, name="mh", tag="mh")
                mg = fpool.tile([128, PAIRW], bf16, name="mg", tag="mg")
                nc.vector.tensor_max(mh[0:rows, :], lo[0:rows, :], hi[0:rows, :])
                nc.vector.tensor_scalar(
                    mg[0:rows, :], mh[0:rows, :], 1.0, None,
                    op0=AluOpType.mult, op1=AluOpType.add,
                    accum_out=acc[0:rows, gi: gi + 1],
                )

            pending = []
            group = []
            gi = 0
            for pi in range(NPAIR):
                pair = CHUNKS[2 * pi: 2 * pi + 2]
                PC = ps.tile([128, 4 * 512], f32, name="PC", tag="bp")
                PM = ps.tile([128, 4 * 512], f32, name="PM", tag="bp")
                for tile_i, bp in enumerate((PC, PM)):
                    for s in range(2):
                        p = 2 * tile_i + s
                        for k, (jd, h0, hcnt) in enumerate(pair):
                            n = hcnt * JW
                            nc.tensor.matmul(
                                bp[:, :].rearrange(
                                    "p (s n) -> p s n", s=4, n=512
                                )[:, 2 * k + s, 0:n],
                                wtv[:, p, :, :],
                                xv[:, jd: jd + 2, h0: h0 + hcnt, 0:JW],
                                start=True,
                                stop=True,
                                perf_mode=mybir.MatmulPerfMode.DoubleRow,
                            )
                mm = mp.tile([128, PAIRW], bf16, name="mm", tag="mm")
                consume_pair(pi, PC, PM, mm)
                group.append(mm)
                if len(group) == 2 or pi == NPAIR - 1:
                    pending.append((gi, group))
                    group = []
                    gi += 1
                    while len(pending) > (1 if pi < NPAIR - 1 else 0):
                        fold_group(*pending.pop(0))

            # total: sum acc columns per partition half, fold halves via DMA.
            tot = ap.tile([128, 1], f32, tag="tot")
            nc.vector.reduce_sum(tot[:, :], acc[:, :], axis=mybir.AxisListType.X)
            tots = ap.tile([OUT_C, 1], f32, tag="tots")
            nc.sync.dma_start(out=tots[:, :], in_=tot[OUT_C:128, :])
            tot2 = ap.tile([OUT_C, 1], f32, tag="tot2")
            nc.vector.tensor_add(tot2[:, :], tot[0:OUT_C, :], tots[:, :])
            res = ap.tile([OUT_C, 1], f32, tag="res")
            # res = tot2 * (SCALE/(NPOS*WS)) + 0.5*b; bvec is pre-scaled.
            nc.vector.scalar_tensor_tensor(
                res[:, :], tot2[:, :], SCALE / (NPOS * WS), bv[:, :],
                op0=AluOpType.mult, op1=AluOpType.add,
            )
            out_t = ap.tile([OUT_C, 1], f32, tag="outt")
            nc.vector.tensor_scalar(
                out_t[:, :], res[:, :], 0.0, 1.0,
                op0=AluOpType.max, op1=AluOpType.min,
            )
            nc.sync.dma_start(out=o_d[:, :], in_=out_t[:, :])

    return nc


_NC_CACHE = None


def _get_nc():
    global _NC_CACHE
    if _NC_CACHE is None:
        _NC_CACHE = build_nc()
        _NC_CACHE.finalize()
    return _NC_CACHE


def run(x, w, b, **spmd_kwargs):
    """Run on 8 cores; returns (output (8,64,1,1,1), BassKernelResults)."""
    import ml_dtypes
    f8 = np.dtype(ml_dtypes.float8_e4m3)
    x = np.ascontiguousarray(x, np.float32)
    wstk = build_wstack(np.asarray(w, np.float32)).astype(f8)
    bvec = (SCALE * np.asarray(b, np.float32)).reshape(OUT_C, 1).copy()
    nc = _get_nc()
    in_maps = [
        {"x": x[i].reshape(IN_C, FREE).astype(f8), "wstk": wstk, "bvec": bvec}
        for i in range(N_BATCH)
    ]
    r = run_bass_kernel_spmd(nc, in_maps, list(range(N_BATCH)), **spmd_kwargs)
    out = np.stack(
        [np.asarray(r.results[i]["out"], np.float32).reshape(OUT_C) for i in range(N_BATCH)]
    )
    return out.reshape(N_BATCH, OUT_C, 1, 1, 1), r


def kernel(x, w, b):
    out, _ = run(x, w, b)
    return out


# revision 7
# speedup vs baseline: 1.2346x; 1.1609x over previous
"""Trainium2 Bass kernel: ConvTranspose3d(32->64,k3,s2,p1) + 0.5x + MaxPool3d(2) +
global-avg-pool + clamp(0,1), data-parallel over batch on 8 NeuronCores.

Math: a stride-2 transposed conv splits into 8 parity classes (even/odd output
index per spatial axis); each 2x2x2 maxpool window holds exactly one output of
each class, so maxpool == elementwise max over the 8 class sub-convolutions.

Conv: fp8(e4m3) DoubleRow matmuls contract K=256 per pass = 4 partition blocks
((dh,dw) shifted copies of x) x 32 c_in x 2 ko k-tiles (the dd shift, a d-axis
stride in the rhs AP).  The full {0,1}^3 shift cube is addressable in one
matmul, so each class's complete tap set fits a single pass: 4 passes x
(2 classes per 128-partition PSUM output) cover all 8 classes at 0.5
cycles/row.  Weights are pre-scaled by 128 into fp8 range; the final mean
scale divides it back out.

Consume (max over 8 classes + mean): PSUM is split into four 2-bank tiles per
pair (chunk x {passes 0-1, passes 2-3}) so each tile is freed by a single
~1us op and the matmul/evac/max pipeline stays decoupled.  ACT evacuates the
pass-0/1 tiles; DVE maxes them against the pass-2/3 tiles directly in PSUM
(1x) on mixed pairs, while heavy pairs ACT-evacuate those too and the max
runs all-SBUF at 2x -- the heavy fraction balances ACT vs DVE.  Cross-half
folds batch two pairs via DMA partition-stacking so the fold max + fused
sum-accumulate use all 128 partitions; the high-half running total is
reduced early, overlapping the tail.
"""

import numpy as np

import concourse.bass as bass
import concourse.bacc as bacc
import concourse.mybir as mybir
from concourse.tile import TileContext
from concourse.bass_utils import run_bass_kernel_spmd
from concourse.alu_op_type import AluOpType

# Problem constants (hardcoded per contract)
N_BATCH = 8
IN_C, OUT_C = 32, 64
D, H, W = 16, 32, 32
JD, JH, JW = 15, 31, 31          # pooled output grid
NPOS = JD * JH * JW              # 14415
SCALE = 0.5
WS = 128.0                       # fp8 weight pre-scale
FREE = D * H * W                 # 16384 flat free size per c_in
DSTR, HSTR = H * W, W            # flat strides

BLOCKS = [(0, 0), (0, 1), (1, 0), (1, 1)]          # (dh, dw) per 32-row block
BLOCK_OFF = [dh * HSTR + dw for (dh, dw) in BLOCKS]

CHUNKS = [(jd, h0, hcnt) for jd in range(JD) for (h0, hcnt) in ((0, 16), (16, 15))]
CN = [hcnt * JW for (_, _, hcnt) in CHUNKS]
PAIRW = CN[0] + CN[1]   # 961 cols per pair tile
NPAIR = len(CHUNKS) // 2
NGROUP = (NPAIR + 1) // 2
NPASS = 4

# Classes (pd, ph, pw); pass p computes CLS[p] as PSUM partition halves.
CLS = [
    ((0, 0, 0), (1, 1, 1)),
    ((0, 0, 1), (1, 1, 0)),
    ((0, 1, 0), (1, 0, 1)),
    ((0, 1, 1), (1, 0, 0)),
]

HEAVY_PAIRS = {2, 4, 7, 9, 12}   # pass-2/3 tiles evacuated too; L1 all-SBUF 2x


def build_wstack(w: np.ndarray) -> np.ndarray:
    """Stack torch-layout ConvTranspose3d weights (in,out,kd,kh,kw) into the
    4 DoubleRow lhsT blocks, one [128, 4*2*128] array: rows = 32*block + c_in;
    cols = 256*pass + 128*ko + 64*half + c_out.  Unused slots stay 0."""
    wstk = np.zeros((128, NPASS * 2 * 128), np.float32)
    for p in range(NPASS):
        for half, (pd, ph, pw) in enumerate(CLS[p]):
            for ko in range(2):
                if pd == 0 and ko == 1:
                    continue
                kd = 1 if pd == 0 else 2 - 2 * ko
                for bidx, (dh, dw) in enumerate(BLOCKS):
                    if dh > ph or dw > pw:
                        continue
                    kh = 1 if ph == 0 else 2 - 2 * dh
                    kw = 1 if pw == 0 else 2 - 2 * dw
                    col = p * 256 + ko * 128 + half * 64
                    wstk[32 * bidx: 32 * bidx + 32, col: col + OUT_C] = (
                        w[:, :, kd, kh, kw] * WS
                    )
    return wstk


def build_nc() -> bass.Bass:
    nc = bacc.Bacc()
    f32 = mybir.dt.float32
    bf16 = mybir.dt.bfloat16
    fp8 = mybir.dt.float8e4

    x_d = nc.declare_dram_parameter("x", [IN_C, FREE], fp8, isOutput=False)
    w_d = nc.declare_dram_parameter("wstk", [128, NPASS * 256], fp8, isOutput=False)
    b_d = nc.declare_dram_parameter("bvec", [OUT_C, 1], f32, isOutput=False)
    o_d = nc.declare_dram_parameter("out", [OUT_C, 1], f32, isOutput=True)

    with TileContext(nc) as tc:
        with (
            tc.tile_pool(name="xp", bufs=1) as xp,
            tc.tile_pool(name="wp", bufs=1) as wp,
            tc.tile_pool(name="ps", bufs=4, space="PSUM") as ps,
            tc.tile_pool(name="mp", bufs=4) as mp,
            tc.tile_pool(name="fp", bufs=2) as fpool,
            tc.tile_pool(name="ap", bufs=1) as ap,
        ):
            # Trigger the ACT table load at t=0 so it overlaps the x DMA.
            warm = ap.tile([1, 1], bf16, tag="warm")
            nc.gpsimd.memset(warm[:, :], 0.0)
            nc.scalar.copy(warm[:, :], warm[:, :])

            wt = wp.tile([128, NPASS * 256], fp8, tag="wt")
            nc.sync.dma_start(out=wt[:, :], in_=w_d[:, :])
            bv = wp.tile([OUT_C, 1], f32, tag="bv")
            nc.gpsimd.dma_start(out=bv[:, :], in_=b_d[:, :])

            xbuf = xp.tile([128, FREE], fp8, tag="x")
            # 4 shifted copies; priority slab (d-rows 0-1) per block first so
            # the first chunk pair's matmuls start early.  Split dispatch
            # between HWDGE (sync) and SWDGE (gpsimd) to halve queue serial.
            SLAB = 2 * DSTR
            HALF = FREE // 2
            for bidx, off in enumerate(BLOCK_OFF):
                eng = nc.sync if bidx % 2 == 0 else nc.gpsimd
                eng.dma_start(
                    out=xbuf[32 * bidx: 32 * bidx + 32, 0:SLAB],
                    in_=x_d[:, off: off + SLAB],
                )
            for bidx, off in enumerate(BLOCK_OFF):
                eng = nc.sync if bidx % 2 == 1 else nc.gpsimd
                eng.dma_start(
                    out=xbuf[32 * bidx: 32 * bidx + 32, SLAB:HALF],
                    in_=x_d[:, SLAB + off: off + HALF],
                )
            for bidx, off in enumerate(BLOCK_OFF):
                ln = FREE - HALF - 33
                eng = nc.sync if bidx % 2 == 0 else nc.gpsimd
                eng.dma_start(
                    out=xbuf[32 * bidx: 32 * bidx + 32, HALF: HALF + ln],
                    in_=x_d[:, HALF + off: HALF + off + ln],
                )

            xv = xbuf[:, :].rearrange("p (d h w) -> p d h w", d=D, h=H, w=W)
            wtv = wt[:, :].rearrange("k (p ko m) -> k p ko m", p=NPASS, ko=2)
            # acc column g: partition p<64 = group g's even pair, p>=64 = odd
            acc = ap.tile([128, NGROUP], f32, tag="acc")
            nc.vector.memset(acc[:, :], 0.0)

            def consume_chunk(pi, k, n, o, TC, TM, mm):
                """One chunk's 4 pass-slots (8 classes) -> mm[:, o:o+n]."""
                TCv = TC[:, :].rearrange("p (s n) -> p s n", s=2, n=512)
                TMv = TM[:, :].rearrange("p (s n) -> p s n", s=2, n=512)
                cc = mp.tile([128, 2, 496], bf16, name="cc", tag="cc")
                nc.scalar.copy(cc[:, :, 0:n], TCv[:, :, 0:n])
                m = mp.tile([128, 2, 496], bf16, name="m", tag="m")
                if pi in HEAVY_PAIRS:
                    cm = mp.tile([128, 2, 496], bf16, name="cm", tag="cm")
                    nc.scalar.copy(cm[:, :, 0:n], TMv[:, :, 0:n])
                    nc.vector.tensor_max(m[:, :, 0:n], cc[:, :, 0:n], cm[:, :, 0:n])
                else:
                    nc.vector.tensor_max(m[:, :, 0:n], cc[:, :, 0:n], TMv[:, :, 0:n])
                nc.vector.tensor_max(
                    mm[:, o: o + n], m[:, 0, 0:n], m[:, 1, 0:n]
                )

            def fold_group(gi, mms):
                """Fold 1-2 pairs' cross-half maxes + accumulate.  For a
                2-pair group, pair i's mm tile becomes the lo stack: pair j's
                lo half overwrites i's hi half once the hi stack has it."""
                if len(mms) == 2:
                    mmi, mmj = mms
                    hi = fpool.tile([128, PAIRW], bf16, name="hi", tag="hi")
                    nc.sync.dma_start(out=hi[0:64, :], in_=mmi[OUT_C:128, :])
                    nc.sync.dma_start(out=hi[64:128, :], in_=mmj[OUT_C:128, :])
                    nc.sync.dma_start(out=mmi[OUT_C:128, :], in_=mmj[0:OUT_C, :])
                    lo, rows = mmi, 128
                else:
                    hi = fpool.tile([128, PAIRW], bf16, name="hi", tag="hi")
                    nc.sync.dma_start(out=hi[0:64, :], in_=mms[0][OUT_C:128, :])
                    lo, rows = mms[0], 64
                mh = fpool.tile([128, PAIRW], bf16, name="mh", tag="mh")
                mg = fpool.tile([128, PAIRW], bf16, name="mg", tag="mg")
                nc.vector.tensor_max(mh[0:rows, :], lo[0:rows, :], hi[0:rows, :])
                nc.vector.tensor_scalar(
                    mg[0:rows, :], mh[0:rows, :], 1.0, None,
                    op0=AluOpType.mult, op1=AluOpType.add,
                    accum_out=acc[0:rows, gi: gi + 1],
                )

            pending = []
            group = []
            gi = 0
            for pi in range(NPAIR):
                pair = CHUNKS[2 * pi: 2 * pi + 2]
                # 4 two-bank PSUM tiles: (chunk k) x (C = passes 0-1, M = 2-3)
                T = [
                    [
                        ps.tile([128, 2 * 512], f32, name=f"T{k}{r}", tag="bp")
                        for r in range(2)
                    ]
                    for k in range(2)
                ]
                for k, (jd, h0, hcnt) in enumerate(pair):
                    n = hcnt * JW
                    for p in range(NPASS):
                        nc.tensor.matmul(
                            T[k][p // 2][:, :].rearrange(
                                "p (s n) -> p s n", s=2, n=512
                            )[:, p % 2, 0:n],
                            wtv[:, p, :, :],
                            xv[:, jd: jd + 2, h0: h0 + hcnt, 0:JW],
                            start=True,
                            stop=True,
                            perf_mode=mybir.MatmulPerfMode.DoubleRow,
                        )
                mm = mp.tile([128, PAIRW], bf16, name="mm", tag="mm")
                for k in range(2):
                    n = CN[2 * pi + k]
                    o = 0 if k == 0 else CN[2 * pi]
                    consume_chunk(pi, k, n, o, T[k][0], T[k][1], mm)
                group.append(mm)
                if len(group) == 2 or pi == NPAIR - 1:
                    pending.append((gi, group))
                    group = []
                    gi += 1
                    while len(pending) > (1 if pi < NPAIR - 1 else 0):
                        fold_group(*pending.pop(0))
                if pi == NPAIR - 2:
                    # acc[64:128] is complete after group NGROUP-2 (the final
                    # single-pair group only writes the low half): reduce and
                    # DMA the high half down early, overlapping the tail.
                    while len(pending) > 0:
                        fold_group(*pending.pop(0))
                    toth = ap.tile([OUT_C, 1], f32, tag="toth")
                    nc.vector.reduce_sum(
                        toth[:, :], acc[OUT_C:128, :], axis=mybir.AxisListType.X
                    )
                    tots = ap.tile([OUT_C, 1], f32, tag="tots")
                    nc.sync.dma_start(out=tots[:, :], in_=toth[:, :])

            totl = ap.tile([OUT_C, 1], f32, tag="totl")
            nc.vector.reduce_sum(
                totl[:, :], acc[0:OUT_C, :], axis=mybir.AxisListType.X
            )
            tot2 = ap.tile([OUT_C, 1], f32, tag="tot2")
            nc.vector.tensor_add(tot2[:, :], totl[:, :], tots[:, :])
            res = ap.tile([OUT_C, 1], f32, tag="res")
            # res = tot2 * (SCALE/(NPOS*WS)) + 0.5*b; bvec is pre-scaled.
            nc.vector.scalar_tensor_tensor(
                res[:, :], tot2[:, :], SCALE / (NPOS * WS), bv[:, :],
                op0=AluOpType.mult, op1=AluOpType.add,
            )
            out_t = ap.tile([OUT_C, 1], f32, tag="outt")
            nc.vector.tensor_scalar(
                out_t[:, :], res[:, :], 0.0, 1.0,
                op0=AluOpType.max, op1=AluOpType.min,
            )
            nc.sync.dma_start(out=o_d[:, :], in_=out_t[:, :])

    return nc


_NC_CACHE = None


def _get_nc():
    global _NC_CACHE
    if _NC_CACHE is None:
        _NC_CACHE = build_nc()
        _NC_CACHE.finalize()
    return _NC_CACHE


def run(x, w, b, **spmd_kwargs):
    """Run on 8 cores; returns (output (8,64,1,1,1), BassKernelResults)."""
    import ml_dtypes
    f8 = np.dtype(ml_dtypes.float8_e4m3)
    x = np.ascontiguousarray(x, np.float32)
    wstk = build_wstack(np.asarray(w, np.float32)).astype(f8)
    bvec = (SCALE * np.asarray(b, np.float32)).reshape(OUT_C, 1).copy()
    nc = _get_nc()
    in_maps = [
        {"x": x[i].reshape(IN_C, FREE).astype(f8), "wstk": wstk, "bvec": bvec}
        for i in range(N_BATCH)
    ]
    r = run_bass_kernel_spmd(nc, in_maps, list(range(N_BATCH)), **spmd_kwargs)
    out = np.stack(
        [np.asarray(r.results[i]["out"], np.float32).reshape(OUT_C) for i in range(N_BATCH)]
    )
    return out.reshape(N_BATCH, OUT_C, 1, 1, 1), r


def kernel(x, w, b):
    out, _ = run(x, w, b)
    return out
